# revision 5
# baseline (speedup 1.0000x reference)
"""Causal depthwise temporal conv (K=4) on 8 TRN2 NeuronCores.

Reference semantics (for x: [B, T, D], w: [K, D], b: [D]):
    out[bt, t, d] = sum_{j=0}^{K-1} x_pad[bt, t + j, d] * w[j, d] + b[d]
where x_pad is x left-padded with K-1 zeros along time.

Strategy:
  - Tensor-parallel over the channel axis: core m owns channels
    [m*512, (m+1)*512) -- the conv is depthwise so channels are fully
    independent (no collectives).
  - Host pre-transposes each core's shard to channel-major [D_sh, B, T+K-1]
    (left zero-padded). On device, channels sit on SBUF partitions so the
    per-channel weight becomes a per-partition scalar, and the temporal
    shifts become free-dimension slices.
  - Per (channel-block, batch): one ACT op computes w0*x + b, then three
    DVE scalar_tensor_tensor ops each fuse (x_shift * w_j) + acc.
    All DMAs are large contiguous HWDGE transfers.
"""

import numpy as np

import concourse.bacc as bacc
import concourse.mybir as mybir
from concourse.tile import TileContext
from concourse import bass_utils

B = 4            # batch
T = 4096         # sequence length
D = 4096         # channels (width)
K = 4            # temporal taps
N_CORES = 8
D_SH = D // N_CORES          # 512 channels per core
P = 128                      # SBUF partitions
N_BLK = D_SH // P            # 4 channel blocks per core
TP = T + K - 1               # padded time length


def _build(b=B, t=T, n_blk=N_BLK, batch_pair=2):
    nc = bacc.Bacc("TRN2")
    tp = t + K - 1
    f32 = mybir.dt.float32
    f16 = mybir.dt.float16
    x = nc.dram_tensor("x", [n_blk, P, b, tp], f16, kind="ExternalInput")
    wb = nc.dram_tensor("wb", [n_blk, P, K + 1], f32, kind="ExternalInput")
    out = nc.dram_tensor("out", [n_blk, P, b, t], f16, kind="ExternalOutput")
    mult, add = mybir.AluOpType.mult, mybir.AluOpType.add
    ident_fn = mybir.ActivationFunctionType.Identity

    with TileContext(nc) as tc:
        with tc.tile_pool(name="pool", bufs=4) as pool, \
             tc.tile_pool(name="poola", bufs=3) as poola:
            for blk in range(n_blk):
                wt = pool.tile([P, K + 1], f32, tag="wb")
                nc.sync.dma_start(wt[:], wb[blk])
                for bb in range(b):
                    # The first and last chains run at half width so the
                    # DVE stream starts earlier (smaller first load+ACT)
                    # and the final store drains sooner.
                    edge = (blk == 0 and bb == 0) or \
                           (blk == n_blk - 1 and bb == b - 1)
                    if edge and t % 2 == 0 and t >= 2048:
                        hw_ = t // 2
                        for h in (0, hw_):
                            Xh = pool.tile([P, hw_ + K - 1], f16, tag="x")
                            nc.sync.dma_start(
                                Xh[:], x[blk, :, bb, h:h + hw_ + K - 1])
                            a0 = poola.tile([P, hw_], f16, tag="accA")
                            nc.scalar.activation(a0[:], Xh[:, 0:hw_],
                                                 ident_fn,
                                                 bias=wt[:, K:K + 1],
                                                 scale=wt[:, 0:1])
                            a1 = poola.tile([P, hw_], f16, tag="accB")
                            nc.vector.scalar_tensor_tensor(
                                a1[:], Xh[:, 1:1 + hw_], wt[:, 1:2], a0[:],
                                mult, add)
                            a2 = poola.tile([P, hw_], f16, tag="accA")
                            nc.vector.scalar_tensor_tensor(
                                a2[:], Xh[:, 2:2 + hw_], wt[:, 2:3], a1[:],
                                mult, add)
                            a3 = poola.tile([P, hw_], f16, tag="accB")
                            nc.vector.scalar_tensor_tensor(
                                a3[:], Xh[:, 3:3 + hw_], wt[:, 3:4], a2[:],
                                mult, add)
                            nc.sync.dma_start(
                                out[blk, :, bb, h:h + hw_], a3[:])
                        continue
                    # Per-batch loads (2.1MB) shorten the pipeline ramp;
                    # bufs=4 keeps several loads in flight.
                    X = pool.tile([P, tp], f16, tag="x")
                    nc.sync.dma_start(X[:], x[blk, :, bb, :])
                    # Per-batch chain, ping-pong accumulators:
                    # ACT does w0*x0+b, DVE does 3 fused FMAs.
                    a0 = poola.tile([P, t], f16, tag="accA")
                    nc.scalar.activation(a0[:], X[:, 0:t], ident_fn,
                                         bias=wt[:, K:K + 1],
                                         scale=wt[:, 0:1])
                    a1 = poola.tile([P, t], f16, tag="accB")
                    nc.vector.scalar_tensor_tensor(
                        a1[:], X[:, 1:1 + t], wt[:, 1:2], a0[:],
                        mult, add)
                    a2 = poola.tile([P, t], f16, tag="accA")
                    nc.vector.scalar_tensor_tensor(
                        a2[:], X[:, 2:2 + t], wt[:, 2:3], a1[:],
                        mult, add)
                    a3 = poola.tile([P, t], f16, tag="accB")
                    nc.vector.scalar_tensor_tensor(
                        a3[:], X[:, 3:3 + t], wt[:, 3:4], a2[:],
                        mult, add)
                    nc.sync.dma_start(out[blk, :, bb, :], a3[:])
    nc.compile()
    return nc


def _prepare(x, w, b):
    x = np.asarray(x, dtype=np.float32)
    w = np.asarray(w, dtype=np.float32)
    b = np.asarray(b, dtype=np.float32)
    # channel-major, left zero-padded time: [D, B, TP], fp16 on the wire
    # (the 2e-2 correctness gate leaves ~30x headroom over fp16's ~7e-4).
    xp = np.zeros((D, B, TP), dtype=np.float16)
    xp[:, :, K - 1:] = x.transpose(2, 0, 1)
    wbt = np.concatenate([w.T, b[:, None]], axis=1).astype(np.float32)  # [D, K+1]
    in_maps = []
    for m in range(N_CORES):
        sl = slice(m * D_SH, (m + 1) * D_SH)
        in_maps.append({
            "x": np.ascontiguousarray(xp[sl]).reshape(N_BLK, P, B, TP),
            "wb": np.ascontiguousarray(wbt[sl]).reshape(N_BLK, P, K + 1),
        })
    return in_maps


def _collect(results):
    out = np.empty((B, T, D), dtype=np.float32)
    for m in range(N_CORES):
        o = np.asarray(results[m]["out"]).astype(np.float32).reshape(D_SH, B, T)
        out[:, :, m * D_SH:(m + 1) * D_SH] = o.transpose(1, 2, 0)
    return out


def _run(in_maps, trace=False, **kwargs):
    nc = _build()
    return bass_utils.run_bass_kernel_spmd(
        nc, in_maps, core_ids=list(range(N_CORES)), trace=trace, **kwargs)


def kernel(x, w, b):
    in_maps = _prepare(x, w, b)
    try:
        res = _run(in_maps)
    except Exception:
        # Transient NRT device errors have been observed on a cold first
        # execute; one retry (fresh compile dir) clears them.
        res = _run(in_maps)
    return _collect(res.results)



# revision 7
# speedup vs baseline: 2.1906x; 2.1906x over previous
"""Causal depthwise temporal conv (K=4) on 8 TRN2 NeuronCores.

Reference semantics (for x: [B, T, D], w: [K, D], b: [D]):
    out[bt, t, d] = sum_{j=0}^{K-1} x_pad[bt, t + j, d] * w[j, d] + b[d]
where x_pad is x left-padded with K-1 zeros along time.

Strategy (memory-bound problem; DMA is the floor at ~100us/core):
  - Tensor-parallel over channels: core m owns channels [m*512, (m+1)*512);
    depthwise conv => fully independent, no collectives.
  - fp16 on the wire (x in, out back) halves HBM traffic vs f32. The 2e-2
    correctness gate leaves ~50x headroom over the resulting ~3e-4 error.
  - Work is spread so no compute engine exceeds the DMA floor. DVE cannot
    run scalar_tensor_tensor chains above 1 elem/cycle on TRN2 (measured:
    fp16 STT = 1x mode, 4.8us per 4096-elem op), so the 4-tap FMA is
    restructured around PSUM accumulation:
      * ACT prefills psum with tap0: w0*x + b  (scale+bias activation)
      * PE  accumulates taps 1-3 as diagonal-stationary matmuls
        (diag(w_j) @ x_shifted adds w_j[ch]*x[ch, t+j] into psum)
      * DVE evicts psum -> SBUF fp16 (its only job, 1x copy)
    Per 2048-col psum region: ACT ~2.0us | PE 12 matmuls ~2.8us |
    DVE ~2.3us | DMA ~3.2us  => DMA-paced pipeline.
"""

import numpy as np

import concourse.bacc as bacc
import concourse.mybir as mybir
from concourse.tile import TileContext
from concourse import bass_utils

B = 4            # batch
T = 4096         # sequence length
D = 4096         # channels (width)
K = 4            # temporal taps
N_CORES = 8
D_SH = D // N_CORES          # 512 channels per core
P = 128                      # SBUF partitions
N_BLK = D_SH // P            # 4 channel blocks per core
TP = T + K - 1               # padded time length
RG = 2048                    # psum region width (4 banks)
MM = 512                     # matmul moving width (1 bank)


def _build(b=B, t=T, n_blk=N_BLK):
    nc = bacc.Bacc("TRN2")
    tp = t + K - 1
    f32 = mybir.dt.float32
    f16 = mybir.dt.float16
    x = nc.dram_tensor("x", [n_blk, P, b, tp], f16, kind="ExternalInput")
    # per-block ACT params: [:, 0]=w0 scale, [:, 1]=bias
    wt = nc.dram_tensor("wt", [n_blk, P, 2], f32, kind="ExternalInput")
    # per-block stationary diagonals for taps 1..3: [P, 3*P] fp16
    wd = nc.dram_tensor("wd", [n_blk, P, (K - 1) * P], f16,
                        kind="ExternalInput")
    out = nc.dram_tensor("out", [n_blk, P, b, t], f16, kind="ExternalOutput")
    ident_fn = mybir.ActivationFunctionType.Identity

    with TileContext(nc) as tc:
        with tc.tile_pool(name="xp", bufs=4) as xp, \
             tc.tile_pool(name="wp", bufs=2) as wp, \
             tc.tile_pool(name="op", bufs=4) as op, \
             tc.psum_pool(name="pp", bufs=2) as pp:
            for blk in range(n_blk):
                wdt = wp.tile([P, (K - 1) * P], f16, tag="wd")
                nc.sync.dma_start(wdt[:], wd[blk])
                wtt = wp.tile([P, 2], f32, tag="wt")
                nc.sync.dma_start(wtt[:], wt[blk])
                for bb in range(b):
                    X = xp.tile([P, tp], f16, tag="x")
                    nc.sync.dma_start(X[:], x[blk, :, bb, :])
                    for c in range(0, t, RG):
                        # tap 0 (+bias) on ACT, independent of the psum chain
                        # (avoids a cross-engine psum RMW race with PE).
                        a0 = op.tile([P, RG], f16, tag="a0")
                        nc.scalar.activation(a0[:], X[:, c:c + RG],
                                             ident_fn,
                                             bias=wtt[:, 1:2],
                                             scale=wtt[:, 0:1])
                        # taps 1..3 accumulate in psum via diag matmuls;
                        # PE owns the banks from reset (start=True at j=1).
                        ps = pp.tile([P, RG], f32, tag="ps")
                        for j in range(1, K):
                            dg = wdt[:, (j - 1) * P:j * P]
                            for k in range(0, RG, MM):
                                nc.tensor.matmul(
                                    ps[:, k:k + MM], dg,
                                    X[:, c + j + k:c + j + k + MM],
                                    start=(j == 1), stop=(j == K - 1))
                        # evict = fused combine: out = psum(taps 1-3) + a0
                        o = op.tile([P, RG], f16, tag="o")
                        nc.vector.tensor_tensor(
                            o[:], ps[:], a0[:], mybir.AluOpType.add)
                        nc.sync.dma_start(out[blk, :, bb, c:c + RG], o[:])
    nc.compile()
    return nc


def _prepare(x, w, b):
    x = np.asarray(x, dtype=np.float32)
    w = np.asarray(w, dtype=np.float32)
    b = np.asarray(b, dtype=np.float32)
    # channel-major, left zero-padded time: [D, B, TP], fp16 on the wire
    xp = np.zeros((D, B, TP), dtype=np.float16)
    xp[:, :, K - 1:] = x.transpose(2, 0, 1)
    # ACT params: scale = w[0], bias = b
    wbt = np.stack([w[0], b], axis=1).astype(np.float32)        # [D, 2]
    # stationary diagonals: wdall[d, (j-1)*P + m] = w[j, d] iff m == d%P
    wdall = np.zeros((D, (K - 1) * P), dtype=np.float16)
    for j in range(1, K):
        cols = (j - 1) * P + (np.arange(D) % P)
        wdall[np.arange(D), cols] = w[j].astype(np.float16)
    in_maps = []
    for m in range(N_CORES):
        sl = slice(m * D_SH, (m + 1) * D_SH)
        in_maps.append({
            "x": np.ascontiguousarray(xp[sl]).reshape(N_BLK, P, B, TP),
            "wt": np.ascontiguousarray(wbt[sl]).reshape(N_BLK, P, 2),
            "wd": np.ascontiguousarray(wdall[sl]).reshape(N_BLK, P, (K - 1) * P),
        })
    return in_maps


def _collect(results):
    out = np.empty((B, T, D), dtype=np.float32)
    for m in range(N_CORES):
        o = np.asarray(results[m]["out"]).astype(np.float32).reshape(D_SH, B, T)
        out[:, :, m * D_SH:(m + 1) * D_SH] = o.transpose(1, 2, 0)
    return out


def _run(in_maps, trace=False, **kwargs):
    nc = _build()
    return bass_utils.run_bass_kernel_spmd(
        nc, in_maps, core_ids=list(range(N_CORES)), trace=trace, **kwargs)


def kernel(x, w, b):
    in_maps = _prepare(x, w, b)
    try:
        res = _run(in_maps)
    except Exception:
        # Transient NRT device errors have been observed on a cold first
        # execute; one retry (fresh compile dir) clears them.
        res = _run(in_maps)
    return _collect(res.results)


# revision 9
# speedup vs baseline: 2.5993x; 1.1865x over previous
"""Causal depthwise temporal conv (K=4) on 8 TRN2 NeuronCores.

Reference semantics (for x: [B, T, D], w: [K, D], b: [D]):
    out[bt, t, d] = sum_{j=0}^{K-1} x_pad[bt, t + j, d] * w[j, d] + b[d]
where x_pad is x left-padded with K-1 zeros along time.

Strategy (memory-bound problem; DMA is the floor at ~100us/core):
  - Tensor-parallel over channels: core m owns channels [m*512, (m+1)*512);
    depthwise conv => fully independent, no collectives.
  - fp16 on the wire (x in, out back) halves HBM traffic vs f32. The 2e-2
    correctness gate leaves ~50x headroom over the resulting ~3e-4 error.
  - Work is spread so no compute engine exceeds the DMA floor. DVE cannot
    run scalar_tensor_tensor chains above 1 elem/cycle on TRN2 (measured:
    fp16 STT = 1x mode, 4.8us per 4096-elem op), so the 4-tap FMA is
    restructured around PSUM accumulation:
      * ACT prefills psum with tap0: w0*x + b  (scale+bias activation)
      * PE  accumulates taps 1-3 as diagonal-stationary matmuls
        (diag(w_j) @ x_shifted adds w_j[ch]*x[ch, t+j] into psum)
      * DVE evicts psum -> SBUF fp16 (its only job, 1x copy)
    Per 2048-col psum region: ACT ~2.0us | PE 12 matmuls ~2.8us |
    DVE ~2.3us | DMA ~3.2us  => DMA-paced pipeline.
"""

import numpy as np

import concourse.bacc as bacc
import concourse.mybir as mybir
from concourse.tile import TileContext
from concourse import bass_utils

B = 4            # batch
T = 4096         # sequence length
D = 4096         # channels (width)
K = 4            # temporal taps
N_CORES = 8
D_SH = D // N_CORES          # 512 channels per core
P = 128                      # SBUF partitions
N_BLK = D_SH // P            # 4 channel blocks per core
TP = T + K - 1               # padded time length
RG = 2048                    # psum region width (4 banks)
MM = 512                     # matmul moving width (1 bank)


def _build(b=B, t=T, n_blk=N_BLK):
    nc = bacc.Bacc("TRN2")
    tp = t + K - 1
    f32 = mybir.dt.float32
    f16 = mybir.dt.float16
    x = nc.dram_tensor("x", [n_blk, P, b, tp], f16, kind="ExternalInput")
    # per-block ACT params: [:, 0]=w0 scale, [:, 1]=bias
    wt = nc.dram_tensor("wt", [n_blk, P, 2], f32, kind="ExternalInput")
    # per-block stationary diagonals for taps 1..3: [P, 3*P] fp16
    wd = nc.dram_tensor("wd", [n_blk, P, (K - 1) * P], f16,
                        kind="ExternalInput")
    out = nc.dram_tensor("out", [n_blk, P, b, t], f16, kind="ExternalOutput")
    ident_fn = mybir.ActivationFunctionType.Identity

    with TileContext(nc) as tc:
        with tc.tile_pool(name="xp", bufs=6) as xp, \
             tc.tile_pool(name="wp", bufs=2) as wp, \
             tc.tile_pool(name="op", bufs=6) as op, \
             tc.psum_pool(name="pp", bufs=2) as pp:
            for blk in range(n_blk):
                wdt = wp.tile([P, (K - 1) * P], f16, tag="wd")
                nc.sync.dma_start(wdt[:], wd[blk])
                wtt = wp.tile([P, 2], f32, tag="wt")
                nc.sync.dma_start(wtt[:], wt[blk])
                for bb in range(b):
                    X = xp.tile([P, tp], f16, tag="x")
                    nc.sync.dma_start(X[:], x[blk, :, bb, :])
                    for c in range(0, t, RG):
                        # tap 0 (+bias) on ACT, independent of the psum chain
                        # (avoids a cross-engine psum RMW race with PE).
                        a0 = op.tile([P, RG], f16, tag="a0")
                        nc.scalar.activation(a0[:], X[:, c:c + RG],
                                             ident_fn,
                                             bias=wtt[:, 1:2],
                                             scale=wtt[:, 0:1])
                        # taps 1..3 accumulate in psum via diag matmuls;
                        # PE owns the banks from reset (start=True at j=1).
                        ps = pp.tile([P, RG], f32, tag="ps")
                        for j in range(1, K):
                            dg = wdt[:, (j - 1) * P:j * P]
                            for k in range(0, RG, MM):
                                nc.tensor.matmul(
                                    ps[:, k:k + MM], dg,
                                    X[:, c + j + k:c + j + k + MM],
                                    start=(j == 1), stop=(j == K - 1))
                        # evict = fused combine: out = psum(taps 1-3) + a0
                        o = op.tile([P, RG], f16, tag="o")
                        nc.vector.tensor_tensor(
                            o[:], ps[:], a0[:], mybir.AluOpType.add)
                        # stores ride the ACT HWDGE ring so they can't
                        # head-of-line-block prefetch loads on the SP ring
                        nc.scalar.dma_start(out[blk, :, bb, c:c + RG], o[:])
    nc.compile()
    return nc


def _prepare(x, w, b):
    x = np.asarray(x, dtype=np.float32)
    w = np.asarray(w, dtype=np.float32)
    b = np.asarray(b, dtype=np.float32)
    # channel-major, left zero-padded time: [D, B, TP], fp16 on the wire
    xp = np.zeros((D, B, TP), dtype=np.float16)
    xp[:, :, K - 1:] = x.transpose(2, 0, 1)
    # ACT params: scale = w[0], bias = b
    wbt = np.stack([w[0], b], axis=1).astype(np.float32)        # [D, 2]
    # stationary diagonals: wdall[d, (j-1)*P + m] = w[j, d] iff m == d%P
    wdall = np.zeros((D, (K - 1) * P), dtype=np.float16)
    for j in range(1, K):
        cols = (j - 1) * P + (np.arange(D) % P)
        wdall[np.arange(D), cols] = w[j].astype(np.float16)
    in_maps = []
    for m in range(N_CORES):
        sl = slice(m * D_SH, (m + 1) * D_SH)
        in_maps.append({
            "x": np.ascontiguousarray(xp[sl]).reshape(N_BLK, P, B, TP),
            "wt": np.ascontiguousarray(wbt[sl]).reshape(N_BLK, P, 2),
            "wd": np.ascontiguousarray(wdall[sl]).reshape(N_BLK, P, (K - 1) * P),
        })
    return in_maps


def _collect(results):
    out = np.empty((B, T, D), dtype=np.float32)
    for m in range(N_CORES):
        o = np.asarray(results[m]["out"]).astype(np.float32).reshape(D_SH, B, T)
        out[:, :, m * D_SH:(m + 1) * D_SH] = o.transpose(1, 2, 0)
    return out


def _run(in_maps, trace=False, **kwargs):
    nc = _build()
    return bass_utils.run_bass_kernel_spmd(
        nc, in_maps, core_ids=list(range(N_CORES)), trace=trace, **kwargs)


def kernel(x, w, b):
    in_maps = _prepare(x, w, b)
    try:
        res = _run(in_maps)
    except Exception:
        # Transient NRT device errors have been observed on a cold first
        # execute; one retry (fresh compile dir) clears them.
        res = _run(in_maps)
    return _collect(res.results)
